# revision 26
# baseline (speedup 1.0000x reference)
"""Trainium2 Bass kernel for GCN-biased sparse attention (nn_Attention_37589553775245).

Reference computation (per batch b of 8, one NeuronCore each):
    qkv = x @ w_qkv; q,k,v per head (H=8, DH=64)
    attn = softmax(q k^T / sqrt(DH)) + A_hat        (A_hat = D^-1/2 (ceil(adj)+I) D^-1/2)
    out = (attn @ v) @ w_out + b_out

Sharding: pure batch-parallel across the 8 cores (B=8). A_hat is computed on
host (cheap) and replicated; weights replicated. No collectives.

v2 design notes (vs the fp32r v1 baseline):
  - ALL matmul operands are bf16 (x, w_qkv, w_out, A_hat^T, qkT, v, exp(s)):
    same PE cost (1 cycle/row) but half the DMA bytes and SBUF residency;
    fp32 PSUM accumulation keeps rel-err ~1e-3 (tolerance 2e-2).
  - scores are computed transposed (sT[j,i]) so softmax denominators ride the
    attn@v matmul via ones columns in an augmented V laid out [1 | v | 1]:
    even heads use cols 1:66 (denom = out row 64, partition base 0), odd heads
    use cols 0:65 with PSUM out at partition base 63 (denom row 63, v rows
    64:128) — output lands directly at yE partition base 64, removing the
    partition-shift SWDGE copy the v1 kernel needed for odd heads.
  - unit order is i-chunk-major: all 8 heads of chunk 0, then chunk 1. Chunk-0
    merges + out-projections for rows 0:512 run woven into chunk-1 attention,
    shortening the serial tail.
  - loads are chunked and split across the three DMA queues (SP ring: xT
    d-chunks + bias; ACT ring: w_qkv chunks + w_out; SWDGE: A_hat^T) so the
    first projection matmul is gated by ~2 small transfers, not one 2MB load.
  - PSUM->SBUF copies for qkT/v/yT go on the (otherwise idle) Pool/GPSIMD
    engine; attention-tail outputs are Pool-copied out of PSUM immediately so
    score/output banks recycle without waiting on the reciprocal-broadcast
    DRAM bounce (partition broadcast needs a DRAM-sourced DMA).
  - everything is SBUF-resident simultaneously (bf16 halves footprints), so
    A_hat^T loads up front on the SWDGE ring and A_hat@V units weave into the
    chunk-0 attention stream as early PE filler.
"""

import os
import sys

import numpy as np

for _p in ("/opt/trn_rl_repo", "/root/.axon_site/_ro/trn_rl_repo"):
    if _p not in sys.path and os.path.isdir(_p):
        sys.path.insert(0, _p)

import ml_dtypes  # noqa: E402

import concourse.bass as bass  # noqa: E402
import concourse.mybir as mybir  # noqa: E402
import concourse.tile as tile  # noqa: E402
from concourse import bacc  # noqa: E402
from concourse.bass_utils import run_bass_kernel_spmd  # noqa: E402

B, N, DIM, H, DH = 8, 1024, 512, 8, 64
F = H * DH          # 512, inner dim
NT = N // 128       # 8 n-tiles (also j-tiles)
DT = DIM // 128     # 4 dim-tiles
FT = F // 128       # 4 f-tiles
NC2 = N // 512      # 2 i-chunks of 512
SCALE = DH ** -0.5

F32 = mybir.dt.float32
BF16 = mybir.dt.bfloat16

_PROGRAM = None
_last_in_maps = None


def _build_program(reps=1, qk_copies_pool=True, o_copy_pool=True,
                   s_bufs=2, o_bufs=2, mm_bufs=2, exps_bufs=6,
                   mmdt="bf16", per2_bufs=2, n_warm=14):
    MDT = {"bf16": BF16, "f32r": mybir.dt.float32r}[mmdt]
    nc = bacc.Bacc("TRN2", target_bir_lowering=False, debug=False, num_devices=8)

    xT_d = nc.dram_tensor("xT", [DIM, N], MDT, kind="ExternalInput")
    wqkv_d = nc.dram_tensor("wqkv", [DIM, 3 * F], MDT, kind="ExternalInput")
    ahatT_d = nc.dram_tensor("ahatT", [N, N], MDT, kind="ExternalInput")
    wout_d = nc.dram_tensor("wout", [F, DIM], MDT, kind="ExternalInput")
    bout_d = nc.dram_tensor("bout", [1, DIM], F32, kind="ExternalInput")
    out_d = nc.dram_tensor("out", [N, DIM], F32, kind="ExternalOutput")

    with tile.TileContext(nc) as tc:
        with (
            nc.allow_low_precision(
                reason="bf16 softmax-normalize/merge; fp32 PSUM accumulation "
                       "everywhere it matters, tolerance is 2e-2"),
            tc.tile_pool(name="per2", bufs=per2_bufs) as per2,
            tc.tile_pool(name="per1", bufs=1) as per1,
            tc.tile_pool(name="exps", bufs=exps_bufs) as exps,
            tc.tile_pool(name="small", bufs=3) as small,
            tc.tile_pool(name="outs", bufs=3) as outs,
            tc.tile_pool(name="dscr", bufs=4, space="DRAM") as dscr,
            tc.tile_pool(name="ps_mm", bufs=mm_bufs, space="PSUM") as ps_mm,
            tc.tile_pool(name="ps_s", bufs=s_bufs, space="PSUM") as ps_s,
            tc.tile_pool(name="ps_o", bufs=o_bufs, space="PSUM") as ps_o,
        ):
          for _rep in range(reps):
            # ---- persistent SBUF tensors -------------------------------
            # per2 (double-buffered): early-lifetime tensors, so rep r+1's
            # projections can overlap rep r's attention when reps>1.
            xT = per2.tile([128, DT, N], MDT)           # xT[dim, n]
            wqkv = per2.tile([128, DT, 3 * F], MDT)
            qkT = per2.tile([128, 2 * FT, N], MDT)      # [f, n] f=q(0:512),k(512:1024)
            v_sb = per2.tile([128, NT, F], MDT)         # v[n, f]
            vaug = per2.tile([128, NT, H, DH + 1], MDT)  # [n, h, v|1]
            wout = per1.tile([128, FT, DIM], MDT)
            ahatT = per1.tile([128, NT, N], MDT)
            yT = per1.tile([128, FT, N], MDT)           # Y^T[f, i] (A_hat V part)
            yE = per1.tile([128, FT, N], MDT)           # Y^T (exp-attention part)
            bout_bc = per1.tile([128, DIM], F32)

            ones_sb = per1.tile([128, 512], BF16)
            # only the ones-column needs initializing (v copies fill the rest)
            nc.gpsimd.memset(vaug[:, :, :, DH:DH + 1], 1.0)
            nc.gpsimd.memset(ones_sb[64:65, :], 1.0)

            # PE warmup: dependency-free matmuls bridge the initial DMA wait
            # so the tensor engine enters the real stream already ramped to
            # full p-state instead of idling and restarting at half clock.
            for _w in range(n_warm if _rep == 0 else 0):
                wps = ps_mm.tile([128, 512], F32, tag="mm", name="wps")
                nc.tensor.matmul(wps[0:64, :], ones_sb[64:65, 0:64],
                                 ones_sb[64:65, :])

            # ---- loads: split across the three DMA queues, ordered so the
            # emit_qk(0)/emit_qk(4) gating chunks land first ---------------
            def load_wqkv(fc, eng):
                eng.dma_start(
                    out=wqkv[:, :, fc * 256:(fc + 1) * 256],
                    in_=wqkv_d[:, fc * 256:(fc + 1) * 256].rearrange(
                        "(t p) f -> p t f", p=128),
                )

            def load_xt(dt_i):
                nc.scalar.dma_start(
                    out=xT[:, dt_i, :],
                    in_=xT_d[dt_i * 128:(dt_i + 1) * 128, :],
                )

            # ALL loads ride the ACT ring: its per-rep traffic drains by
            # ~15us, so with double-buffered per2 tiles the NEXT rep's loads
            # descriptor-generate ~100us before that rep's PE stream needs
            # them (the SP ring keeps the mid-rep bounces + stores instead).
            load_xt(0)
            load_wqkv(0, nc.scalar)      # q cols 0:256 (heads 0..3)
            load_wqkv(2, nc.scalar)      # k cols 512:768 (heads 0..3)
            for dt_i in range(1, DT):
                load_xt(dt_i)
            nc.scalar.dma_start(out=bout_bc, in_=bout_d[0:1, :].to_broadcast((128, DIM)))
            for fc in (4, 5, 1, 3):      # v cols first, rest of q/k
                load_wqkv(fc, nc.scalar)
            nc.scalar.dma_start(
                out=wout,
                in_=wout_d[:, :].rearrange("(t p) n -> p t n", p=128),
            )
            # A_hat^T rides the ACT HWDGE ring last: off the critical path,
            # and NOT on the SWDGE/Pool queue (a software-driven SWDGE DMA
            # occupies the Pool sequencer for its whole transfer, which would
            # block the qkT/v PSUM->SBUF Pool copies behind it).
            nc.scalar.dma_start(
                out=ahatT,
                in_=ahatT_d[:, :].rearrange("(t p) n -> p t n", p=128),
            )

            # PSUM->SBUF copies: GPSIMD/Pool cannot access PSUM on real
            # TRN2 (BIR verifier rejects it), so spread them over ACT (idle
            # during the projection phase) and DVE.
            def qk_copy(dst, src, eng="act"):
                if eng == "act" and qk_copies_pool:
                    nc.scalar.copy(out=dst, in_=src)
                else:
                    nc.vector.tensor_copy(out=dst, in_=src)

            def dve_copy(dst, src):
                nc.vector.tensor_copy(out=dst, in_=src)

            # ---- phase 1: qT/kT (transposed) and v (natural) -----------
            def emit_qk(ft, eng="act"):
                for c in range(NC2):
                    ps = ps_mm.tile([128, 512], F32, tag="mm")
                    for dt_i in range(DT):
                        nc.tensor.matmul(
                            ps,
                            wqkv[:, dt_i, ft * 128:(ft + 1) * 128],
                            xT[:, dt_i, c * 512:(c + 1) * 512],
                            start=(dt_i == 0),
                            stop=(dt_i == DT - 1),
                        )
                    qk_copy(qkT[:, ft, c * 512:(c + 1) * 512], ps, eng)

            def emit_v(nt_lo=0, nt_hi=NT):
                for nt in range(nt_lo, nt_hi):
                    ps = ps_mm.tile([128, 512], F32, tag="mm")
                    for dt_i in range(DT):
                        nc.tensor.matmul(
                            ps,
                            xT[:, dt_i, nt * 128:(nt + 1) * 128],
                            wqkv[:, dt_i, 2 * F:3 * F],
                            start=(dt_i == 0),
                            stop=(dt_i == DT - 1),
                        )
                    dve_copy(v_sb[:, nt, :], ps)
                    nc.vector.tensor_copy(
                        out=vaug[:, nt, :, 0:DH],
                        in_=ps.rearrange("p (h d) -> p h d", h=H),
                    )

            def ahat_unit(ft, c):
                # (A_hat @ V)^T [f-tile ft, i-chunk c] -> yT
                ps = ps_mm.tile([128, 512], F32, tag="mm")
                for jt in range(NT):
                    nc.tensor.matmul(
                        ps,
                        v_sb[:, jt, ft * 128:(ft + 1) * 128],
                        ahatT[:, jt, c * 512:(c + 1) * 512],
                        start=(jt == 0),
                        stop=(jt == NT - 1),
                    )
                dve_copy(yT[:, ft, c * 512:(c + 1) * 512], ps)

            def attn_tail_pre(h, po):
                # DVE/Pool part of the softmax normalize: copy the exp-attn
                # rows + ridden denominator out of PSUM (bank rows become
                # dead), take the reciprocal, and for odd heads SWDGE-shift
                # the bf16 rows to partitions 64:128 (to line up with their
                # yE slice; DVE lanes cannot cross partitions).
                osb = small.tile([128, 512], BF16, tag="osb")
                nc.vector.tensor_copy(out=osb[0:65, :], in_=po[0:65, :])
                rc = small.tile([128, 512], BF16, tag="recip")
                nc.vector.reciprocal(out=rc[64:65, :], in_=osb[64:65, :])
                if h % 2 == 1:
                    nc.gpsimd.dma_start(out=osb[64:128, :], in_=osb[0:64, :])
                return osb, rc

            def attn_tail_fin(h, c, po, osb, rc, pe_bcast=False):
                # partition-broadcast of the reciprocal. Default: DRAM bounce
                # (2 chained DMAs, ~4us latency but fully off the engines and
                # pipelined across units). pe_bcast: a K=1 PE matmul
                # (ones[1,64]^T @ rc[1,512]) into the now-dead rows of the
                # same PSUM bank (~213ns latency) — used for the final units
                # where the bounce latency would sit on the critical path.
                vlo, vhi = (64, 128) if h % 2 == 1 else (0, 64)
                ysl = yE[vlo:vhi, h // 2, c * 512:(c + 1) * 512]
                if pe_bcast:
                    nc.tensor.matmul(
                        po[vlo:vhi, :],
                        ones_sb[64:65, 0:64],
                        rc[64:65, :],
                    )
                    nc.vector.tensor_mul(ysl, osb[vlo:vhi, :], po[vlo:vhi, :])
                else:
                    scr = dscr.tile([1, 512], BF16, tag="scr")
                    nc.sync.dma_start(out=scr, in_=rc[64:65, :])
                    bcast = small.tile([128, 512], BF16, tag="bcast")
                    nc.sync.dma_start(out=bcast[vlo:vhi, :],
                                      in_=scr.to_broadcast((64, 512)))
                    nc.vector.tensor_mul(ysl, osb[vlo:vhi, :],
                                         bcast[vlo:vhi, :])

            def scores_jb(h, c, jb):
                # one 2-j-tile score block + its exp; returns the et tile
                hb = (h % 2) * 64
                ht = h // 2
                ps_sc = ps_s.tile([128, 2, 512], F32, tag="ps")
                for e in range(2):
                    jt = jb * 2 + e
                    # scoresT[j, i] = sum_d kT[d, j] qT[d, i]
                    nc.tensor.matmul(
                        ps_sc[:, e, :],
                        qkT[hb:hb + 64, FT + ht, jt * 128:(jt + 1) * 128],
                        qkT[hb:hb + 64, ht, c * 512:(c + 1) * 512],
                    )
                et = exps.tile([128, 2, 512], MDT, tag="exp")
                nc.scalar.activation(
                    out=et, in_=ps_sc,
                    func=mybir.ActivationFunctionType.Exp,
                    scale=float(SCALE),
                )
                return et

            def av_jb(h, out_ap, jb, et):
                for e in range(2):
                    jt = jb * 2 + e
                    # [expv^T ; denom] accumulation
                    nc.tensor.matmul(
                        out_ap,
                        vaug[:, jt, h, :],
                        et[:, e, :],
                        start=(jt == 0),
                        stop=(jt == NT - 1),
                    )

            def attn_unit(h, c, weave=None, pe_bcast=False):
                # one head, one 512-wide i-chunk. Score blocks run one jb
                # ahead of the attn@v accumulation so the PE has independent
                # work while ACT computes each exp batch.
                ps_out = ps_o.tile([128, 512], F32, tag="po")
                out_ap = ps_out[0:65, :]
                ets = [scores_jb(h, c, 0), scores_jb(h, c, 1)]
                av_jb(h, out_ap, 0, ets[0])
                ets.append(scores_jb(h, c, 2))
                av_jb(h, out_ap, 1, ets[1])
                ets.append(scores_jb(h, c, 3))
                av_jb(h, out_ap, 2, ets[2])
                av_jb(h, out_ap, 3, ets[3])
                osb, rc = attn_tail_pre(h, ps_out)
                if weave is not None:
                    weave()
                attn_tail_fin(h, c, ps_out, osb, rc, pe_bcast=pe_bcast)

            def merge(ft, c):
                sl = slice(c * 512, (c + 1) * 512)
                nc.vector.tensor_add(yT[:, ft, sl], yT[:, ft, sl], yE[:, ft, sl])

            def outproj(nt):
                ps = ps_mm.tile([128, 512], F32, tag="mm")
                for ft in range(FT):
                    nc.tensor.matmul(
                        ps,
                        yT[:, ft, nt * 128:(nt + 1) * 128],
                        wout[:, ft, :],
                        start=(ft == 0),
                        stop=(ft == FT - 1),
                    )
                ot = outs.tile([128, DIM], F32, tag="ot")
                nc.vector.tensor_add(ot, ps, bout_bc)
                nc.sync.dma_start(out=out_d[nt * 128:(nt + 1) * 128, :], in_=ot)

            # ---- emission schedule -------------------------------------
            emit_qk(0)        # q heads 0,1
            emit_qk(4)        # k heads 0,1

            # unit (h=0, c=0) emits its score blocks interleaved with emit_v
            # halves so ACT's 64us exp stream starts ~12us earlier; its
            # attn@v runs after emit_v (vaug must precede it in PE order).
            u0_et = [scores_jb(0, 0, 0), scores_jb(0, 0, 1)]
            emit_v(0, NT // 2)
            u0_et += [scores_jb(0, 0, 2), scores_jb(0, 0, 3)]
            emit_v(NT // 2, NT)
            u0_po = ps_o.tile([128, 512], F32, tag="po")
            for jb in range(4):
                av_jb(0, u0_po[0:65, :], jb, u0_et[jb])
            u0_osb, u0_rc = attn_tail_pre(0, u0_po)
            emit_qk(1)        # q heads 2,3 (PE filler under the reciprocal)
            attn_tail_fin(0, 0, u0_po, u0_osb, u0_rc)  # bounce

            # chunk 0: remaining 7 units, weaving the other q/k tiles
            # (PE filler while ACT chews exp) and the chunk-0 A_hat units.
            def weave_c0(h):
                rest_qk = [5, 2, 6, 3, 7]
                def w():
                    if h - 1 < len(rest_qk):
                        emit_qk(rest_qk[h - 1])
                    if h >= 4:
                        ahat_unit(h - 4, 0)
                return w

            for h in range(1, H):
                attn_unit(h, 0, weave=weave_c0(h))
                if h >= 4:
                    merge(h - 4, 0)

            # chunk 1: weave chunk-0 out-projections + chunk-1 A_hat units.
            # outproj(3) is held back as tail filler under the last unit's
            # reciprocal; nt4/nt5 pre-accumulate their ft0..2 partials so
            # only ~10 matmuls remain after the final merge.
            def weave_c1(h):
                def w():
                    if h < 3:
                        outproj(h)
                    else:
                        ahat_unit(h - 4, 1)
                return w

            for h in range(H):
                attn_unit(h, 1, weave=weave_c1(h) if (h < 3 or h >= 4) else None,
                          pe_bcast=(h >= 6))
                if h >= 4:
                    merge(h - 4, 1)

            outproj(3)                       # c0 rows: no merge(.,1) dep
            pp = {}
            for nt in (4, 5):                # partials over merged ft 0..2
                pp[nt] = ps_mm.tile([128, 512], F32, tag="mm", name=f"pp{nt}")
                for ft in range(FT - 1):
                    nc.tensor.matmul(
                        pp[nt],
                        yT[:, ft, nt * 128:(nt + 1) * 128],
                        wout[:, ft, :],
                        start=(ft == 0),
                        stop=False,
                    )
            for nt in (4, 5):                # finish after merge(3,1)
                nc.tensor.matmul(
                    pp[nt],
                    yT[:, FT - 1, nt * 128:(nt + 1) * 128],
                    wout[:, FT - 1, :],
                    start=False,
                    stop=True,
                )
                ot = outs.tile([128, DIM], F32, tag="ot")
                nc.vector.tensor_add(ot, pp[nt], bout_bc)
                nc.sync.dma_start(out=out_d[nt * 128:(nt + 1) * 128, :], in_=ot)
            for nt in (6, 7):
                outproj(nt)

    nc.compile()
    return nc


def _get_program():
    global _PROGRAM
    if _PROGRAM is None:
        _PROGRAM = _build_program()
    return _PROGRAM


def kernel(x, adj, w_qkv, w_out, b_out):
    x = np.asarray(x, dtype=np.float32)
    adj = np.asarray(adj, dtype=np.float32)
    w_qkv = np.asarray(w_qkv, dtype=np.float32)
    w_out = np.asarray(w_out, dtype=np.float32)
    b_out = np.asarray(b_out, dtype=np.float32).reshape(1, DIM)

    # host-side: normalized adjacency bias, replicated (cheap: one 1024^2 pass)
    A = np.ceil(adj) + np.eye(N, dtype=np.float32)
    dinv = A.sum(axis=1) ** -0.5
    A_hat = (A * dinv[:, None]) * dinv[None, :]

    bf = ml_dtypes.bfloat16
    ahatT = np.ascontiguousarray(A_hat.T).astype(bf)
    wqkv_b = np.ascontiguousarray(w_qkv).astype(bf)
    wout_b = np.ascontiguousarray(w_out).astype(bf)

    nc = _get_program()
    in_maps = []
    for b in range(B):
        in_maps.append({
            "xT": np.ascontiguousarray(x[b].T).astype(bf),
            "wqkv": wqkv_b,
            "ahatT": ahatT,
            "wout": wout_b,
            "bout": b_out,
        })
    global _last_in_maps
    _last_in_maps = in_maps
    res = run_bass_kernel_spmd(nc, in_maps, list(range(B)))
    out = np.stack([res.results[b]["out"] for b in range(B)], axis=0)
    return out.astype(np.float32)


if __name__ == "__main__":
    rng = np.random.default_rng(0)
    x = rng.standard_normal((B, N, DIM), dtype=np.float32)
    adj = (rng.random((N, N), dtype=np.float32) < 0.05).astype(np.float32) * 0.5
    w_qkv = rng.standard_normal((DIM, 3 * F), dtype=np.float32) * DIM ** -0.5
    w_out = rng.standard_normal((F, DIM), dtype=np.float32) * F ** -0.5
    b_out = np.zeros(DIM, dtype=np.float32)
    out = kernel(x=x, adj=adj, w_qkv=w_qkv, w_out=w_out, b_out=b_out)
    print("out", out.shape, out.dtype, np.abs(out).max())


# revision 36
# speedup vs baseline: 74.9084x; 74.9084x over previous
"""Trainium2 Bass kernel for GCN-biased sparse attention (nn_Attention_37589553775245).

Reference computation (per batch b of 8, one NeuronCore each):
    qkv = x @ w_qkv; q,k,v per head (H=8, DH=64)
    attn = softmax(q k^T / sqrt(DH)) + A_hat        (A_hat = D^-1/2 (ceil(adj)+I) D^-1/2)
    out = (attn @ v) @ w_out + b_out

Sharding: pure batch-parallel across the 8 cores (B=8). A_hat is computed on
host (cheap) and replicated; weights replicated. No collectives.

v2 design notes (vs the fp32r v1 baseline):
  - ALL matmul operands are bf16 (x, w_qkv, w_out, A_hat^T, qkT, v, exp(s)):
    same PE cost (1 cycle/row) but half the DMA bytes and SBUF residency;
    fp32 PSUM accumulation keeps rel-err ~1e-3 (tolerance 2e-2).
  - scores are computed transposed (sT[j,i]) so softmax denominators ride the
    attn@v matmul via ones columns in an augmented V laid out [1 | v | 1]:
    even heads use cols 1:66 (denom = out row 64, partition base 0), odd heads
    use cols 0:65 with PSUM out at partition base 63 (denom row 63, v rows
    64:128) — output lands directly at yE partition base 64, removing the
    partition-shift SWDGE copy the v1 kernel needed for odd heads.
  - unit order is i-chunk-major: all 8 heads of chunk 0, then chunk 1. Chunk-0
    merges + out-projections for rows 0:512 run woven into chunk-1 attention,
    shortening the serial tail.
  - loads are chunked and split across the three DMA queues (SP ring: xT
    d-chunks + bias; ACT ring: w_qkv chunks + w_out; SWDGE: A_hat^T) so the
    first projection matmul is gated by ~2 small transfers, not one 2MB load.
  - PSUM->SBUF copies for qkT/v/yT go on the (otherwise idle) Pool/GPSIMD
    engine; attention-tail outputs are Pool-copied out of PSUM immediately so
    score/output banks recycle without waiting on the reciprocal-broadcast
    DRAM bounce (partition broadcast needs a DRAM-sourced DMA).
  - everything is SBUF-resident simultaneously (bf16 halves footprints), so
    A_hat^T loads up front on the SWDGE ring and A_hat@V units weave into the
    chunk-0 attention stream as early PE filler.
"""

import os
import sys

import numpy as np

for _p in ("/opt/trn_rl_repo", "/root/.axon_site/_ro/trn_rl_repo"):
    if _p not in sys.path and os.path.isdir(_p):
        sys.path.insert(0, _p)

import ml_dtypes  # noqa: E402

import concourse.bass as bass  # noqa: E402
import concourse.mybir as mybir  # noqa: E402
import concourse.tile as tile  # noqa: E402
from concourse import bacc  # noqa: E402
from concourse.bass_utils import run_bass_kernel_spmd  # noqa: E402

B, N, DIM, H, DH = 8, 1024, 512, 8, 64
F = H * DH          # 512, inner dim
NT = N // 128       # 8 n-tiles (also j-tiles)
DT = DIM // 128     # 4 dim-tiles
FT = F // 128       # 4 f-tiles
NC2 = N // 512      # 2 i-chunks of 512
SCALE = DH ** -0.5

F32 = mybir.dt.float32
BF16 = mybir.dt.bfloat16

_PROGRAM = None
_last_in_maps = None


def _build_program(reps=1, qk_copies_pool=True, o_copy_pool=True,
                   s_bufs=2, o_bufs=2, mm_bufs=2, exps_bufs=6,
                   mmdt="bf16", per2_bufs=2, n_warm=8):
    MDT = {"bf16": BF16, "f32r": mybir.dt.float32r}[mmdt]
    nc = bacc.Bacc("TRN2", target_bir_lowering=False, debug=False, num_devices=8)

    xT_d = nc.dram_tensor("xT", [DIM, N], MDT, kind="ExternalInput")
    wqkv_d = nc.dram_tensor("wqkv", [DIM, 3 * F], MDT, kind="ExternalInput")
    ahatT_d = nc.dram_tensor("ahatT", [N, N], MDT, kind="ExternalInput")
    wout_d = nc.dram_tensor("wout", [F, DIM], MDT, kind="ExternalInput")
    bout_d = nc.dram_tensor("bout", [1, DIM], F32, kind="ExternalInput")
    out_d = nc.dram_tensor("out", [N, DIM], F32, kind="ExternalOutput")

    with tile.TileContext(nc) as tc:
        with (
            nc.allow_low_precision(
                reason="bf16 softmax-normalize/merge; fp32 PSUM accumulation "
                       "everywhere it matters, tolerance is 2e-2"),
            tc.tile_pool(name="per2", bufs=per2_bufs) as per2,
            tc.tile_pool(name="per1", bufs=1) as per1,
            tc.tile_pool(name="exps", bufs=exps_bufs) as exps,
            tc.tile_pool(name="small", bufs=3) as small,
            tc.tile_pool(name="outs", bufs=3) as outs,
            tc.tile_pool(name="dscr", bufs=4, space="DRAM") as dscr,
            tc.tile_pool(name="ps_mm", bufs=mm_bufs, space="PSUM") as ps_mm,
            tc.tile_pool(name="ps_s", bufs=s_bufs, space="PSUM") as ps_s,
            tc.tile_pool(name="ps_o", bufs=o_bufs, space="PSUM") as ps_o,
        ):
          for _rep in range(reps):
            # ---- persistent SBUF tensors -------------------------------
            # per2 (double-buffered): early-lifetime tensors, so rep r+1's
            # projections can overlap rep r's attention when reps>1.
            xT = per2.tile([128, DT, N], MDT)           # xT[dim, n]
            wqkv = per2.tile([128, DT, 3 * F], MDT)
            qkT = per2.tile([128, 2 * FT, N], MDT)      # [f, n] f=q(0:512),k(512:1024)
            v_sb = per2.tile([128, NT, F], MDT)         # v[n, f]
            vaug = per2.tile([128, NT, H, DH + 1], MDT)  # [n, h, v|1]
            wout = per1.tile([128, FT, DIM], MDT)
            ahatT = per1.tile([128, NT, N], MDT)
            yT = per1.tile([128, FT, N], MDT)           # Y^T[f, i] (A_hat V part)
            yE = per1.tile([128, FT, N], MDT)           # Y^T (exp-attention part)
            bout_bc = per1.tile([128, DIM], F32)

            ones_sb = per1.tile([128, 512], BF16)
            # only the ones-column needs initializing (v copies fill the rest)
            nc.gpsimd.memset(vaug[:, :, :, DH:DH + 1], 1.0)
            nc.gpsimd.memset(ones_sb[64:65, :], 1.0)

            # PE warmup: dependency-free matmuls bridge the initial DMA wait
            # so the tensor engine enters the real stream already ramped to
            # full p-state instead of idling and restarting at half clock.
            for _w in range(n_warm if _rep == 0 else 0):
                wps = ps_mm.tile([128, 512], F32, tag="mm", name="wps")
                nc.tensor.matmul(wps[0:64, :], ones_sb[64:65, 0:64],
                                 ones_sb[64:65, :])

            # ---- loads: split across the three DMA queues, ordered so the
            # emit_qk(0)/emit_qk(4) gating chunks land first ---------------
            def load_wqkv(fc, eng):
                eng.dma_start(
                    out=wqkv[:, :, fc * 256:(fc + 1) * 256],
                    in_=wqkv_d[:, fc * 256:(fc + 1) * 256].rearrange(
                        "(t p) f -> p t f", p=128),
                )

            def load_xt(dt_i):
                nc.scalar.dma_start(
                    out=xT[:, dt_i, :],
                    in_=xT_d[dt_i * 128:(dt_i + 1) * 128, :],
                )

            # ALL loads ride the ACT ring: its per-rep traffic drains by
            # ~15us, so with double-buffered per2 tiles the NEXT rep's loads
            # descriptor-generate ~100us before that rep's PE stream needs
            # them (the SP ring keeps the mid-rep bounces + stores instead).
            load_xt(0)
            load_wqkv(0, nc.scalar)      # q cols 0:256 (heads 0..3)
            for dt_i in range(1, DT):
                load_xt(dt_i)
            load_wqkv(2, nc.scalar)      # k cols 512:768 (heads 0..3)
            for fc in (4, 5, 1, 3):      # v cols first, rest of q/k
                load_wqkv(fc, nc.scalar)
            nc.scalar.dma_start(
                out=wout,
                in_=wout_d[:, :].rearrange("(t p) n -> p t n", p=128),
            )
            # A_hat^T rides the ACT HWDGE ring last: off the critical path,
            # and NOT on the SWDGE/Pool queue (a software-driven SWDGE DMA
            # occupies the Pool sequencer for its whole transfer, which would
            # block the qkT/v PSUM->SBUF Pool copies behind it).
            nc.scalar.dma_start(
                out=ahatT,
                in_=ahatT_d[:, :].rearrange("(t p) n -> p t n", p=128),
            )
            nc.scalar.dma_start(out=bout_bc, in_=bout_d[0:1, :].to_broadcast((128, DIM)))

            # PSUM->SBUF copies: GPSIMD/Pool cannot access PSUM on real
            # TRN2 (BIR verifier rejects it), so spread them over ACT (idle
            # during the projection phase) and DVE.
            def qk_copy(dst, src, eng="act"):
                if eng == "act" and qk_copies_pool:
                    nc.scalar.copy(out=dst, in_=src)
                else:
                    nc.vector.tensor_copy(out=dst, in_=src)

            def dve_copy(dst, src):
                nc.vector.tensor_copy(out=dst, in_=src)

            # ---- phase 1: qT/kT (transposed) and v (natural) -----------
            def emit_qk(ft, eng="act"):
                for c in range(NC2):
                    ps = ps_mm.tile([128, 512], F32, tag="mm")
                    for dt_i in range(DT):
                        nc.tensor.matmul(
                            ps,
                            wqkv[:, dt_i, ft * 128:(ft + 1) * 128],
                            xT[:, dt_i, c * 512:(c + 1) * 512],
                            start=(dt_i == 0),
                            stop=(dt_i == DT - 1),
                        )
                    qk_copy(qkT[:, ft, c * 512:(c + 1) * 512], ps, eng)

            def emit_v(nt_lo=0, nt_hi=NT):
                for nt in range(nt_lo, nt_hi):
                    ps = ps_mm.tile([128, 512], F32, tag="mm")
                    for dt_i in range(DT):
                        nc.tensor.matmul(
                            ps,
                            xT[:, dt_i, nt * 128:(nt + 1) * 128],
                            wqkv[:, dt_i, 2 * F:3 * F],
                            start=(dt_i == 0),
                            stop=(dt_i == DT - 1),
                        )
                    dve_copy(v_sb[:, nt, :], ps)
                    nc.vector.tensor_copy(
                        out=vaug[:, nt, :, 0:DH],
                        in_=ps.rearrange("p (h d) -> p h d", h=H),
                    )

            def ahat_unit(ft, c, mid=None):
                # (A_hat @ V)^T [f-tile ft, i-chunk c] -> yT. `mid` lets the
                # caller interleave work (e.g. the attention tail finish)
                # halfway through the accumulation so merges are not delayed
                # by the full 8-matmul chain.
                ps = ps_mm.tile([128, 512], F32, tag="mm")
                for jt in range(NT):
                    if jt == NT // 2 and mid is not None:
                        mid()
                    nc.tensor.matmul(
                        ps,
                        v_sb[:, jt, ft * 128:(ft + 1) * 128],
                        ahatT[:, jt, c * 512:(c + 1) * 512],
                        start=(jt == 0),
                        stop=(jt == NT - 1),
                    )
                dve_copy(yT[:, ft, c * 512:(c + 1) * 512], ps)

            def attn_tail_pre(h, po):
                # DVE/Pool part of the softmax normalize: copy the exp-attn
                # rows + ridden denominator out of PSUM (bank rows become
                # dead), take the reciprocal, and for odd heads SWDGE-shift
                # the bf16 rows to partitions 64:128 (to line up with their
                # yE slice; DVE lanes cannot cross partitions).
                osb = small.tile([128, 512], BF16, tag="osb")
                nc.vector.tensor_copy(out=osb[0:65, :], in_=po[0:65, :])
                rc = small.tile([128, 512], BF16, tag="recip")
                nc.vector.reciprocal(out=rc[64:65, :], in_=osb[64:65, :])
                if h % 2 == 1:
                    nc.gpsimd.dma_start(out=osb[64:128, :], in_=osb[0:64, :])
                return osb, rc

            def attn_tail_fin(h, c, po, osb, rc, pe_bcast=False):
                # partition-broadcast of the reciprocal. Default: DRAM bounce
                # (2 chained DMAs, ~4us latency but fully off the engines and
                # pipelined across units). pe_bcast: a K=1 PE matmul
                # (ones[1,64]^T @ rc[1,512]) into the now-dead rows of the
                # same PSUM bank (~213ns latency) — used for the final units
                # where the bounce latency would sit on the critical path.
                vlo, vhi = (64, 128) if h % 2 == 1 else (0, 64)
                ysl = yE[vlo:vhi, h // 2, c * 512:(c + 1) * 512]
                if pe_bcast:
                    nc.tensor.matmul(
                        po[vlo:vhi, :],
                        ones_sb[64:65, 0:64],
                        rc[64:65, :],
                    )
                    nc.vector.tensor_mul(ysl, osb[vlo:vhi, :], po[vlo:vhi, :])
                else:
                    scr = dscr.tile([1, 512], BF16, tag="scr")
                    nc.sync.dma_start(out=scr, in_=rc[64:65, :])
                    bcast = small.tile([128, 512], BF16, tag="bcast")
                    nc.sync.dma_start(out=bcast[vlo:vhi, :],
                                      in_=scr.to_broadcast((64, 512)))
                    nc.vector.tensor_mul(ysl, osb[vlo:vhi, :],
                                         bcast[vlo:vhi, :])

            def scores_jb(h, c, jb):
                # one 2-j-tile score block + its exp; returns the et tile
                hb = (h % 2) * 64
                ht = h // 2
                ps_sc = ps_s.tile([128, 2, 512], F32, tag="ps")
                for e in range(2):
                    jt = jb * 2 + e
                    # scoresT[j, i] = sum_d kT[d, j] qT[d, i]
                    nc.tensor.matmul(
                        ps_sc[:, e, :],
                        qkT[hb:hb + 64, FT + ht, jt * 128:(jt + 1) * 128],
                        qkT[hb:hb + 64, ht, c * 512:(c + 1) * 512],
                    )
                et = exps.tile([128, 2, 512], MDT, tag="exp")
                nc.scalar.activation(
                    out=et, in_=ps_sc,
                    func=mybir.ActivationFunctionType.Exp,
                    scale=float(SCALE),
                )
                return et

            def av_jb(h, out_ap, jb, et):
                for e in range(2):
                    jt = jb * 2 + e
                    # [expv^T ; denom] accumulation
                    nc.tensor.matmul(
                        out_ap,
                        vaug[:, jt, h, :],
                        et[:, e, :],
                        start=(jt == 0),
                        stop=(jt == NT - 1),
                    )

            def attn_unit(h, c, weave=None, pe_bcast=False):
                # one head, one 512-wide i-chunk. Score blocks run one jb
                # ahead of the attn@v accumulation so the PE has independent
                # work while ACT computes each exp batch.
                ps_out = ps_o.tile([128, 512], F32, tag="po")
                out_ap = ps_out[0:65, :]
                ets = [scores_jb(h, c, 0), scores_jb(h, c, 1)]
                av_jb(h, out_ap, 0, ets[0])
                ets.append(scores_jb(h, c, 2))
                av_jb(h, out_ap, 1, ets[1])
                ets.append(scores_jb(h, c, 3))
                av_jb(h, out_ap, 2, ets[2])
                av_jb(h, out_ap, 3, ets[3])
                osb, rc = attn_tail_pre(h, ps_out)
                if weave is not None:
                    weave()
                attn_tail_fin(h, c, ps_out, osb, rc, pe_bcast=pe_bcast)

            def merge(ft, c):
                sl = slice(c * 512, (c + 1) * 512)
                nc.vector.tensor_add(yT[:, ft, sl], yT[:, ft, sl], yE[:, ft, sl])

            def outproj(nt):
                ps = ps_mm.tile([128, 512], F32, tag="mm")
                for ft in range(FT):
                    nc.tensor.matmul(
                        ps,
                        yT[:, ft, nt * 128:(nt + 1) * 128],
                        wout[:, ft, :],
                        start=(ft == 0),
                        stop=(ft == FT - 1),
                    )
                ot = outs.tile([128, DIM], F32, tag="ot")
                nc.vector.tensor_add(ot, ps, bout_bc)
                nc.sync.dma_start(out=out_d[nt * 128:(nt + 1) * 128, :], in_=ot)

            # ---- emission schedule -------------------------------------
            emit_qk(0)        # q heads 0,1
            emit_qk(4)        # k heads 0,1

            # unit (h=0, c=0) emits its score blocks interleaved with emit_v
            # halves so ACT's 64us exp stream starts ~12us earlier; its
            # attn@v runs after emit_v (vaug must precede it in PE order).
            u0_et = [scores_jb(0, 0, 0), scores_jb(0, 0, 1)]
            emit_v(0, NT // 2)
            u0_et += [scores_jb(0, 0, 2), scores_jb(0, 0, 3)]
            emit_v(NT // 2, NT)
            u0_po = ps_o.tile([128, 512], F32, tag="po")
            for jb in range(4):
                av_jb(0, u0_po[0:65, :], jb, u0_et[jb])
            u0_osb, u0_rc = attn_tail_pre(0, u0_po)
            emit_qk(1)        # q heads 2,3 (PE filler under the reciprocal)
            attn_tail_fin(0, 0, u0_po, u0_osb, u0_rc)  # bounce

            # remaining 15 units, software-pipelined: each unit's first two
            # score blocks (sc0/sc1) are emitted inside the PREVIOUS unit, so
            # a unit's attn@v never starts cold on ACT, and ACT's exp stream
            # stays fed through the endgame. weave_a (qk-tile emission) must
            # precede the next unit's scores; weave_b (A_hat / outproj
            # filler) sits between tail_pre and tail_fin to cover the
            # reciprocal latency.
            rest_qk = [5, 2, 6, 3, 7]
            units = []
            for h in range(1, H):
                units.append(dict(
                    h=h, c=0,
                    weave_a=(lambda ft: (lambda: emit_qk(ft)))(
                        rest_qk[h - 1]) if h - 1 < len(rest_qk) else None,
                    weave_b=(lambda ft: (lambda fin: ahat_unit(
                        ft, 0, mid=lambda: fin.pop()())))(
                        h - 4) if h >= 4 else None,
                    post=(lambda ft: (lambda: merge(ft, 0)))(
                        h - 4) if h >= 4 else None,
                    pe_bcast=False,
                ))
            for h in range(H):
                wb = None
                if h < 3:
                    wb = (lambda nt: (lambda fin: outproj(nt)))(h)
                elif h >= 4:
                    wb = (lambda ft: (lambda fin: ahat_unit(
                        ft, 1, mid=lambda: fin.pop()())))(h - 4)
                units.append(dict(
                    h=h, c=1, weave_a=None, weave_b=wb,
                    post=(lambda ft: (lambda: merge(ft, 1)))(
                        h - 4) if h >= 4 else None,
                    pe_bcast=(h >= 6),
                ))

            def emit_units(units, first_ets):
                ets = {0: first_ets}           # unit idx -> [et tiles]
                for i, u in enumerate(units):
                    h, c = u["h"], u["c"]
                    po = ps_o.tile([128, 512], F32, tag="po", name="po")
                    out_ap = po[0:65, :]
                    e = ets.pop(i)
                    av_jb(h, out_ap, 0, e[0])
                    e.append(scores_jb(h, c, 2))
                    av_jb(h, out_ap, 1, e[1])
                    e.append(scores_jb(h, c, 3))
                    if u["weave_a"] is not None:
                        u["weave_a"]()
                    av_jb(h, out_ap, 2, e[2])
                    nxt = units[i + 1] if i + 1 < len(units) else None
                    if nxt is not None:
                        ets[i + 1] = [scores_jb(nxt["h"], nxt["c"], 0)]
                    av_jb(h, out_ap, 3, e[3])
                    if nxt is not None:
                        ets[i + 1].append(scores_jb(nxt["h"], nxt["c"], 1))
                    osb, rc = attn_tail_pre(h, po)
                    fin = [lambda: attn_tail_fin(h, c, po, osb, rc,
                                                 pe_bcast=u["pe_bcast"])]
                    if u["weave_b"] is not None:
                        u["weave_b"](fin)
                    if fin:
                        fin.pop()()
                    if u["post"] is not None:
                        u["post"]()

            emit_units(units, first_ets=[scores_jb(1, 0, 0), scores_jb(1, 0, 1)])

            outproj(3)                       # c0 rows: no merge(.,1) dep
            pp = {}
            for nt in (4, 5):                # partials over merged ft 0..2
                pp[nt] = ps_mm.tile([128, 512], F32, tag="mm", name=f"pp{nt}")
                for ft in range(FT - 1):
                    nc.tensor.matmul(
                        pp[nt],
                        yT[:, ft, nt * 128:(nt + 1) * 128],
                        wout[:, ft, :],
                        start=(ft == 0),
                        stop=False,
                    )
            for nt in (4, 5):                # finish after merge(3,1)
                nc.tensor.matmul(
                    pp[nt],
                    yT[:, FT - 1, nt * 128:(nt + 1) * 128],
                    wout[:, FT - 1, :],
                    start=False,
                    stop=True,
                )
                ot = outs.tile([128, DIM], F32, tag="ot")
                nc.vector.tensor_add(ot, pp[nt], bout_bc)
                nc.sync.dma_start(out=out_d[nt * 128:(nt + 1) * 128, :], in_=ot)
            for nt in (6, 7):
                outproj(nt)

    nc.compile()
    return nc


def _get_program():
    global _PROGRAM
    if _PROGRAM is None:
        _PROGRAM = _build_program()
    return _PROGRAM


def kernel(x, adj, w_qkv, w_out, b_out):
    x = np.asarray(x, dtype=np.float32)
    adj = np.asarray(adj, dtype=np.float32)
    w_qkv = np.asarray(w_qkv, dtype=np.float32)
    w_out = np.asarray(w_out, dtype=np.float32)
    b_out = np.asarray(b_out, dtype=np.float32).reshape(1, DIM)

    # host-side: normalized adjacency bias, replicated (cheap: one 1024^2 pass)
    A = np.ceil(adj) + np.eye(N, dtype=np.float32)
    dinv = A.sum(axis=1) ** -0.5
    A_hat = (A * dinv[:, None]) * dinv[None, :]

    bf = ml_dtypes.bfloat16
    ahatT = np.ascontiguousarray(A_hat.T).astype(bf)
    wqkv_b = np.ascontiguousarray(w_qkv).astype(bf)
    wout_b = np.ascontiguousarray(w_out).astype(bf)

    nc = _get_program()
    in_maps = []
    for b in range(B):
        in_maps.append({
            "xT": np.ascontiguousarray(x[b].T).astype(bf),
            "wqkv": wqkv_b,
            "ahatT": ahatT,
            "wout": wout_b,
            "bout": b_out,
        })
    global _last_in_maps
    _last_in_maps = in_maps
    res = run_bass_kernel_spmd(nc, in_maps, list(range(B)))
    out = np.stack([res.results[b]["out"] for b in range(B)], axis=0)
    return out.astype(np.float32)


if __name__ == "__main__":
    rng = np.random.default_rng(0)
    x = rng.standard_normal((B, N, DIM), dtype=np.float32)
    adj = (rng.random((N, N), dtype=np.float32) < 0.05).astype(np.float32) * 0.5
    w_qkv = rng.standard_normal((DIM, 3 * F), dtype=np.float32) * DIM ** -0.5
    w_out = rng.standard_normal((F, DIM), dtype=np.float32) * F ** -0.5
    b_out = np.zeros(DIM, dtype=np.float32)
    out = kernel(x=x, adj=adj, w_qkv=w_qkv, w_out=w_out, b_out=b_out)
    print("out", out.shape, out.dtype, np.abs(out).max())


# revision 38
# speedup vs baseline: 83.0006x; 1.1080x over previous
"""Trainium2 Bass kernel for GCN-biased sparse attention (nn_Attention_37589553775245).

Reference computation (per batch b of 8, one NeuronCore each):
    qkv = x @ w_qkv; q,k,v per head (H=8, DH=64)
    attn = softmax(q k^T / sqrt(DH)) + A_hat        (A_hat = D^-1/2 (ceil(adj)+I) D^-1/2)
    out = (attn @ v) @ w_out + b_out

Sharding: pure batch-parallel across the 8 cores (B=8). A_hat is computed on
host (cheap) and replicated; weights replicated. No collectives.

Design (TimelineSim ~116.6us single-shot / ~111us marginal per rep; PE busy
~99us is the bound — measured HW tracks sim at ~1.4x):
  - ALL matmul operands are bf16 (fp32 PSUM accumulation): same PE cost as
    fp32r at free-dim 512 (1 cycle/row) but half the DMA bytes and SBUF
    residency. rel-err ~5e-3 vs the 2e-2 gate. fp8 was evaluated and
    rejected: quantizing q/k makes a per-row systematic score shift softmax
    cannot average away, and v noise is amplified through the A_hat path.
  - scores are computed transposed (sT[j,i] via lhsT=kT, rhs=qT, K=d=64) so
    softmax denominators ride the attn@v matmul via a ones column in the
    augmented V ([v|1]): PSUM out rows 0:64 = head values, row 64 = denom.
    Odd heads SWDGE-shift their bf16 copy to partitions 64:128 to line up
    with their yE slice (DVE lanes cannot cross partitions; PE matmuls can
    only target PSUM partition bases 0/32/64).
  - the reciprocal 1/denom is partition-broadcast via a DRAM bounce for most
    units (2 chained DMAs, fully off the engines, pipelined across units) and
    via a K=1 ones^T@recip PE matmul into the dead rows of the same PSUM bank
    for the final two units, where bounce latency would sit on the critical
    path.
  - 16 attention units (8 heads x 2 i-chunks) run i-chunk-major and
    SOFTWARE-PIPELINED: each unit's first two score blocks are emitted inside
    the previous unit, so attn@v never starts cold on ACT's exp stream (ACT
    is the second-busiest engine at ~78us). Remaining q/k projection tiles,
    A_hat@V units (split around the attention-tail finish), and the chunk-0
    out-projections weave into the stream as PE filler.
  - out-projections for rows 512:768 pre-accumulate their first 3 f-tiles so
    only ~10 matmuls remain after the final merge; dependency-free warmup
    matmuls bridge the initial DMA wait so the PE enters the real stream at
    full p-state.
  - ALL loads ride the ACT HWDGE ring (drained by ~15us), stores + bounce
    DMAs ride the SP ring: with double-buffered input tiles, rep r+1's loads
    prefetch ~100us early when the program is built with reps>1, and neither
    ring head-of-line-blocks the other. A_hat^T must NOT ride the SWDGE ring
    (a SWDGE DMA occupies the Pool sequencer for its whole transfer).
  - PSUM->SBUF copies go on ACT early (projection phase, ACT idle) and DVE
    elsewhere; GPSIMD/Pool cannot access PSUM on real TRN2 (BIR verifier
    rejects it; CoreSim does not catch it).
  - PSUM budget (8 banks): 2 projection/output accumulators + 2x2-bank score
    tiles + 2 attention-output banks.
"""

import os
import sys

import numpy as np

for _p in ("/opt/trn_rl_repo", "/root/.axon_site/_ro/trn_rl_repo"):
    if _p not in sys.path and os.path.isdir(_p):
        sys.path.insert(0, _p)

import ml_dtypes  # noqa: E402

import concourse.bass as bass  # noqa: E402
import concourse.mybir as mybir  # noqa: E402
import concourse.tile as tile  # noqa: E402
from concourse import bacc  # noqa: E402
from concourse.bass_utils import run_bass_kernel_spmd  # noqa: E402

B, N, DIM, H, DH = 8, 1024, 512, 8, 64
F = H * DH          # 512, inner dim
NT = N // 128       # 8 n-tiles (also j-tiles)
DT = DIM // 128     # 4 dim-tiles
FT = F // 128       # 4 f-tiles
NC2 = N // 512      # 2 i-chunks of 512
SCALE = DH ** -0.5

F32 = mybir.dt.float32
BF16 = mybir.dt.bfloat16

_PROGRAM = None
_last_in_maps = None


def _build_program(reps=1, qk_copies_pool=True, o_copy_pool=True,
                   s_bufs=2, o_bufs=2, mm_bufs=2, exps_bufs=6,
                   mmdt="bf16", per2_bufs=2, n_warm=8, small_bufs=3):
    MDT = {"bf16": BF16, "f32r": mybir.dt.float32r}[mmdt]
    nc = bacc.Bacc("TRN2", target_bir_lowering=False, debug=False, num_devices=8)

    xT_d = nc.dram_tensor("xT", [DIM, N], MDT, kind="ExternalInput")
    wqkv_d = nc.dram_tensor("wqkv", [DIM, 3 * F], MDT, kind="ExternalInput")
    ahatT_d = nc.dram_tensor("ahatT", [N, N], MDT, kind="ExternalInput")
    wout_d = nc.dram_tensor("wout", [F, DIM], MDT, kind="ExternalInput")
    bout_d = nc.dram_tensor("bout", [1, DIM], F32, kind="ExternalInput")
    out_d = nc.dram_tensor("out", [N, DIM], F32, kind="ExternalOutput")

    with tile.TileContext(nc) as tc:
        with (
            nc.allow_low_precision(
                reason="bf16 softmax-normalize/merge; fp32 PSUM accumulation "
                       "everywhere it matters, tolerance is 2e-2"),
            tc.tile_pool(name="per2", bufs=per2_bufs) as per2,
            tc.tile_pool(name="per1", bufs=1) as per1,
            tc.tile_pool(name="exps", bufs=exps_bufs) as exps,
            tc.tile_pool(name="small", bufs=small_bufs) as small,
            tc.tile_pool(name="outs", bufs=3) as outs,
            tc.tile_pool(name="dscr", bufs=4, space="DRAM") as dscr,
            tc.tile_pool(name="ps_mm", bufs=mm_bufs, space="PSUM") as ps_mm,
            tc.tile_pool(name="ps_s", bufs=s_bufs, space="PSUM") as ps_s,
            tc.tile_pool(name="ps_o", bufs=o_bufs, space="PSUM") as ps_o,
        ):
          for _rep in range(reps):
            # ---- persistent SBUF tensors -------------------------------
            # per2 (double-buffered): early-lifetime tensors, so rep r+1's
            # projections can overlap rep r's attention when reps>1.
            xT = per2.tile([128, DT, N], MDT)           # xT[dim, n]
            wqkv = per2.tile([128, DT, 3 * F], MDT)
            qkT = per2.tile([128, 2 * FT, N], MDT)      # [f, n] f=q(0:512),k(512:1024)
            v_sb = per2.tile([128, NT, F], MDT)         # v[n, f]
            vaug = per2.tile([128, NT, H, DH + 1], MDT)  # [n, h, v|1]
            wout = per1.tile([128, FT, DIM], MDT)
            ahatT = per1.tile([128, NT, N], MDT)
            yT = per1.tile([128, FT, N], MDT)           # Y^T[f, i] (A_hat V part)
            yE = per1.tile([128, FT, N], MDT)           # Y^T (exp-attention part)
            bout_bc = per1.tile([128, DIM], F32)

            ones_sb = per1.tile([128, 512], BF16)
            # only the ones-column needs initializing (v copies fill the rest)
            nc.gpsimd.memset(vaug[:, :, :, DH:DH + 1], 1.0)
            nc.gpsimd.memset(ones_sb[64:65, :], 1.0)

            # PE warmup: dependency-free matmuls bridge the initial DMA wait
            # so the tensor engine enters the real stream already ramped to
            # full p-state instead of idling and restarting at half clock.
            for _w in range(n_warm if _rep == 0 else 0):
                wps = ps_mm.tile([128, 512], F32, tag="mm", name="wps")
                nc.tensor.matmul(wps[0:64, :], ones_sb[64:65, 0:64],
                                 ones_sb[64:65, :])

            # ---- loads: split across the three DMA queues, ordered so the
            # emit_qk(0)/emit_qk(4) gating chunks land first ---------------
            def load_wqkv(fc, eng):
                eng.dma_start(
                    out=wqkv[:, :, fc * 256:(fc + 1) * 256],
                    in_=wqkv_d[:, fc * 256:(fc + 1) * 256].rearrange(
                        "(t p) f -> p t f", p=128),
                )

            def load_xt(dt_i):
                nc.scalar.dma_start(
                    out=xT[:, dt_i, :],
                    in_=xT_d[dt_i * 128:(dt_i + 1) * 128, :],
                )

            # ALL loads ride the ACT ring: its per-rep traffic drains by
            # ~15us, so with double-buffered per2 tiles the NEXT rep's loads
            # descriptor-generate ~100us before that rep's PE stream needs
            # them (the SP ring keeps the mid-rep bounces + stores instead).
            load_xt(0)
            load_wqkv(0, nc.scalar)      # q cols 0:256 (heads 0..3)
            for dt_i in range(1, DT):
                load_xt(dt_i)
            load_wqkv(2, nc.scalar)      # k cols 512:768 (heads 0..3)
            for fc in (4, 5, 1, 3):      # v cols first, rest of q/k
                load_wqkv(fc, nc.scalar)
            nc.scalar.dma_start(
                out=wout,
                in_=wout_d[:, :].rearrange("(t p) n -> p t n", p=128),
            )
            # A_hat^T rides the ACT HWDGE ring last: off the critical path,
            # and NOT on the SWDGE/Pool queue (a software-driven SWDGE DMA
            # occupies the Pool sequencer for its whole transfer, which would
            # block the qkT/v PSUM->SBUF Pool copies behind it).
            nc.scalar.dma_start(
                out=ahatT,
                in_=ahatT_d[:, :].rearrange("(t p) n -> p t n", p=128),
            )
            nc.scalar.dma_start(out=bout_bc, in_=bout_d[0:1, :].to_broadcast((128, DIM)))

            # PSUM->SBUF copies: GPSIMD/Pool cannot access PSUM on real
            # TRN2 (BIR verifier rejects it), so spread them over ACT (idle
            # during the projection phase) and DVE.
            def qk_copy(dst, src, eng="act"):
                if eng == "act" and qk_copies_pool:
                    nc.scalar.copy(out=dst, in_=src)
                else:
                    nc.vector.tensor_copy(out=dst, in_=src)

            def dve_copy(dst, src):
                nc.vector.tensor_copy(out=dst, in_=src)

            # ---- phase 1: qT/kT (transposed) and v (natural) -----------
            def emit_qk(ft, eng="act"):
                for c in range(NC2):
                    ps = ps_mm.tile([128, 512], F32, tag="mm")
                    for dt_i in range(DT):
                        nc.tensor.matmul(
                            ps,
                            wqkv[:, dt_i, ft * 128:(ft + 1) * 128],
                            xT[:, dt_i, c * 512:(c + 1) * 512],
                            start=(dt_i == 0),
                            stop=(dt_i == DT - 1),
                        )
                    qk_copy(qkT[:, ft, c * 512:(c + 1) * 512], ps, eng)

            def emit_v(nt_lo=0, nt_hi=NT):
                for nt in range(nt_lo, nt_hi):
                    ps = ps_mm.tile([128, 512], F32, tag="mm")
                    for dt_i in range(DT):
                        nc.tensor.matmul(
                            ps,
                            xT[:, dt_i, nt * 128:(nt + 1) * 128],
                            wqkv[:, dt_i, 2 * F:3 * F],
                            start=(dt_i == 0),
                            stop=(dt_i == DT - 1),
                        )
                    dve_copy(v_sb[:, nt, :], ps)
                    nc.vector.tensor_copy(
                        out=vaug[:, nt, :, 0:DH],
                        in_=ps.rearrange("p (h d) -> p h d", h=H),
                    )

            def ahat_unit(ft, c, mid=None):
                # (A_hat @ V)^T [f-tile ft, i-chunk c] -> yT. `mid` lets the
                # caller interleave work (e.g. the attention tail finish)
                # halfway through the accumulation so merges are not delayed
                # by the full 8-matmul chain.
                ps = ps_mm.tile([128, 512], F32, tag="mm")
                for jt in range(NT):
                    if jt == NT // 2 and mid is not None:
                        mid()
                    nc.tensor.matmul(
                        ps,
                        v_sb[:, jt, ft * 128:(ft + 1) * 128],
                        ahatT[:, jt, c * 512:(c + 1) * 512],
                        start=(jt == 0),
                        stop=(jt == NT - 1),
                    )
                dve_copy(yT[:, ft, c * 512:(c + 1) * 512], ps)

            def attn_tail_pre(h, po):
                # DVE/Pool part of the softmax normalize: copy the exp-attn
                # rows + ridden denominator out of PSUM (bank rows become
                # dead), take the reciprocal, and for odd heads SWDGE-shift
                # the bf16 rows to partitions 64:128 (to line up with their
                # yE slice; DVE lanes cannot cross partitions).
                osb = small.tile([128, 512], BF16, tag="osb")
                nc.vector.tensor_copy(out=osb[0:65, :], in_=po[0:65, :])
                rc = small.tile([128, 512], BF16, tag="recip")
                nc.vector.reciprocal(out=rc[64:65, :], in_=osb[64:65, :])
                if h % 2 == 1:
                    nc.gpsimd.dma_start(out=osb[64:128, :], in_=osb[0:64, :])
                return osb, rc

            def attn_tail_fin(h, c, po, osb, rc, pe_bcast=False):
                # partition-broadcast of the reciprocal. Default: DRAM bounce
                # (2 chained DMAs, ~4us latency but fully off the engines and
                # pipelined across units). pe_bcast: a K=1 PE matmul
                # (ones[1,64]^T @ rc[1,512]) into the now-dead rows of the
                # same PSUM bank (~213ns latency) — used for the final units
                # where the bounce latency would sit on the critical path.
                vlo, vhi = (64, 128) if h % 2 == 1 else (0, 64)
                ysl = yE[vlo:vhi, h // 2, c * 512:(c + 1) * 512]
                if pe_bcast:
                    nc.tensor.matmul(
                        po[vlo:vhi, :],
                        ones_sb[64:65, 0:64],
                        rc[64:65, :],
                    )
                    nc.vector.tensor_mul(ysl, osb[vlo:vhi, :], po[vlo:vhi, :])
                else:
                    scr = dscr.tile([1, 512], BF16, tag="scr")
                    nc.sync.dma_start(out=scr, in_=rc[64:65, :])
                    bcast = small.tile([128, 512], BF16, tag="bcast")
                    nc.sync.dma_start(out=bcast[vlo:vhi, :],
                                      in_=scr.to_broadcast((64, 512)))
                    nc.vector.tensor_mul(ysl, osb[vlo:vhi, :],
                                         bcast[vlo:vhi, :])

            def scores_jb(h, c, jb):
                # one 2-j-tile score block + its exp; returns the et tile
                hb = (h % 2) * 64
                ht = h // 2
                ps_sc = ps_s.tile([128, 2, 512], F32, tag="ps")
                for e in range(2):
                    jt = jb * 2 + e
                    # scoresT[j, i] = sum_d kT[d, j] qT[d, i]
                    nc.tensor.matmul(
                        ps_sc[:, e, :],
                        qkT[hb:hb + 64, FT + ht, jt * 128:(jt + 1) * 128],
                        qkT[hb:hb + 64, ht, c * 512:(c + 1) * 512],
                    )
                et = exps.tile([128, 2, 512], MDT, tag="exp")
                nc.scalar.activation(
                    out=et, in_=ps_sc,
                    func=mybir.ActivationFunctionType.Exp,
                    scale=float(SCALE),
                )
                return et

            def av_jb(h, out_ap, jb, et):
                for e in range(2):
                    jt = jb * 2 + e
                    # [expv^T ; denom] accumulation
                    nc.tensor.matmul(
                        out_ap,
                        vaug[:, jt, h, :],
                        et[:, e, :],
                        start=(jt == 0),
                        stop=(jt == NT - 1),
                    )

            def attn_unit(h, c, weave=None, pe_bcast=False):
                # one head, one 512-wide i-chunk. Score blocks run one jb
                # ahead of the attn@v accumulation so the PE has independent
                # work while ACT computes each exp batch.
                ps_out = ps_o.tile([128, 512], F32, tag="po")
                out_ap = ps_out[0:65, :]
                ets = [scores_jb(h, c, 0), scores_jb(h, c, 1)]
                av_jb(h, out_ap, 0, ets[0])
                ets.append(scores_jb(h, c, 2))
                av_jb(h, out_ap, 1, ets[1])
                ets.append(scores_jb(h, c, 3))
                av_jb(h, out_ap, 2, ets[2])
                av_jb(h, out_ap, 3, ets[3])
                osb, rc = attn_tail_pre(h, ps_out)
                if weave is not None:
                    weave()
                attn_tail_fin(h, c, ps_out, osb, rc, pe_bcast=pe_bcast)

            def merge(ft, c):
                sl = slice(c * 512, (c + 1) * 512)
                nc.vector.tensor_add(yT[:, ft, sl], yT[:, ft, sl], yE[:, ft, sl])

            def outproj(nt):
                ps = ps_mm.tile([128, 512], F32, tag="mm")
                for ft in range(FT):
                    nc.tensor.matmul(
                        ps,
                        yT[:, ft, nt * 128:(nt + 1) * 128],
                        wout[:, ft, :],
                        start=(ft == 0),
                        stop=(ft == FT - 1),
                    )
                ot = outs.tile([128, DIM], F32, tag="ot")
                nc.vector.tensor_add(ot, ps, bout_bc)
                nc.sync.dma_start(out=out_d[nt * 128:(nt + 1) * 128, :], in_=ot)

            # ---- emission schedule -------------------------------------
            emit_qk(0)        # q heads 0,1
            emit_qk(4)        # k heads 0,1

            # unit (h=0, c=0) emits its score blocks interleaved with emit_v
            # halves so ACT's 64us exp stream starts ~12us earlier; its
            # attn@v runs after emit_v (vaug must precede it in PE order).
            u0_et = [scores_jb(0, 0, 0), scores_jb(0, 0, 1)]
            emit_v(0, NT // 2)
            u0_et += [scores_jb(0, 0, 2), scores_jb(0, 0, 3)]
            emit_v(NT // 2, NT)
            u0_po = ps_o.tile([128, 512], F32, tag="po")
            for jb in range(4):
                av_jb(0, u0_po[0:65, :], jb, u0_et[jb])
            u0_osb, u0_rc = attn_tail_pre(0, u0_po)
            emit_qk(1)        # q heads 2,3 (PE filler under the reciprocal)
            attn_tail_fin(0, 0, u0_po, u0_osb, u0_rc)  # bounce

            # remaining 15 units, software-pipelined: each unit's first two
            # score blocks (sc0/sc1) are emitted inside the PREVIOUS unit, so
            # a unit's attn@v never starts cold on ACT, and ACT's exp stream
            # stays fed through the endgame. weave_a (qk-tile emission) must
            # precede the next unit's scores; weave_b (A_hat / outproj
            # filler) sits between tail_pre and tail_fin to cover the
            # reciprocal latency.
            rest_qk = [5, 2, 6, 3, 7]
            units = []
            for h in range(1, H):
                units.append(dict(
                    h=h, c=0,
                    weave_a=(lambda ft: (lambda: emit_qk(ft)))(
                        rest_qk[h - 1]) if h - 1 < len(rest_qk) else None,
                    weave_b=(lambda ft: (lambda fin: ahat_unit(
                        ft, 0, mid=lambda: fin.pop()())))(
                        h - 4) if h >= 4 else None,
                    post=(lambda ft: (lambda: merge(ft, 0)))(
                        h - 4) if h >= 4 else None,
                    pe_bcast=False,
                ))
            for h in range(H):
                wb = None
                if h < 3:
                    wb = (lambda nt: (lambda fin: outproj(nt)))(h)
                elif h >= 4:
                    wb = (lambda ft: (lambda fin: ahat_unit(
                        ft, 1, mid=lambda: fin.pop()())))(h - 4)
                units.append(dict(
                    h=h, c=1, weave_a=None, weave_b=wb,
                    post=(lambda ft: (lambda: merge(ft, 1)))(
                        h - 4) if h >= 4 else None,
                    pe_bcast=(h >= 6),
                ))

            def emit_units(units, first_ets):
                ets = {0: first_ets}           # unit idx -> [et tiles]
                for i, u in enumerate(units):
                    h, c = u["h"], u["c"]
                    po = ps_o.tile([128, 512], F32, tag="po", name="po")
                    out_ap = po[0:65, :]
                    e = ets.pop(i)
                    av_jb(h, out_ap, 0, e[0])
                    e.append(scores_jb(h, c, 2))
                    av_jb(h, out_ap, 1, e[1])
                    e.append(scores_jb(h, c, 3))
                    if u["weave_a"] is not None:
                        u["weave_a"]()
                    av_jb(h, out_ap, 2, e[2])
                    nxt = units[i + 1] if i + 1 < len(units) else None
                    if nxt is not None:
                        ets[i + 1] = [scores_jb(nxt["h"], nxt["c"], 0)]
                    av_jb(h, out_ap, 3, e[3])
                    if nxt is not None:
                        ets[i + 1].append(scores_jb(nxt["h"], nxt["c"], 1))
                    osb, rc = attn_tail_pre(h, po)
                    fin = [lambda: attn_tail_fin(h, c, po, osb, rc,
                                                 pe_bcast=u["pe_bcast"])]
                    if u["weave_b"] is not None:
                        u["weave_b"](fin)
                    if fin:
                        fin.pop()()
                    if u["post"] is not None:
                        u["post"]()

            emit_units(units, first_ets=[scores_jb(1, 0, 0), scores_jb(1, 0, 1)])

            outproj(3)                       # c0 rows: no merge(.,1) dep
            pp = {}
            for nt in (4, 5):                # partials over merged ft 0..2
                pp[nt] = ps_mm.tile([128, 512], F32, tag="mm", name=f"pp{nt}")
                for ft in range(FT - 1):
                    nc.tensor.matmul(
                        pp[nt],
                        yT[:, ft, nt * 128:(nt + 1) * 128],
                        wout[:, ft, :],
                        start=(ft == 0),
                        stop=False,
                    )
            for nt in (4, 5):                # finish after merge(3,1)
                nc.tensor.matmul(
                    pp[nt],
                    yT[:, FT - 1, nt * 128:(nt + 1) * 128],
                    wout[:, FT - 1, :],
                    start=False,
                    stop=True,
                )
                ot = outs.tile([128, DIM], F32, tag="ot")
                nc.vector.tensor_add(ot, pp[nt], bout_bc)
                nc.sync.dma_start(out=out_d[nt * 128:(nt + 1) * 128, :], in_=ot)
            for nt in (6, 7):
                outproj(nt)

    nc.compile()
    return nc


def _get_program():
    global _PROGRAM
    if _PROGRAM is None:
        _PROGRAM = _build_program()
    return _PROGRAM


def kernel(x, adj, w_qkv, w_out, b_out):
    x = np.asarray(x, dtype=np.float32)
    adj = np.asarray(adj, dtype=np.float32)
    w_qkv = np.asarray(w_qkv, dtype=np.float32)
    w_out = np.asarray(w_out, dtype=np.float32)
    b_out = np.asarray(b_out, dtype=np.float32).reshape(1, DIM)

    # host-side: normalized adjacency bias, replicated (cheap: one 1024^2 pass)
    A = np.ceil(adj) + np.eye(N, dtype=np.float32)
    dinv = A.sum(axis=1) ** -0.5
    A_hat = (A * dinv[:, None]) * dinv[None, :]

    bf = ml_dtypes.bfloat16
    ahatT = np.ascontiguousarray(A_hat.T).astype(bf)
    wqkv_b = np.ascontiguousarray(w_qkv).astype(bf)
    wout_b = np.ascontiguousarray(w_out).astype(bf)

    nc = _get_program()
    in_maps = []
    for b in range(B):
        in_maps.append({
            "xT": np.ascontiguousarray(x[b].T).astype(bf),
            "wqkv": wqkv_b,
            "ahatT": ahatT,
            "wout": wout_b,
            "bout": b_out,
        })
    global _last_in_maps
    _last_in_maps = in_maps
    res = run_bass_kernel_spmd(nc, in_maps, list(range(B)))
    out = np.stack([res.results[b]["out"] for b in range(B)], axis=0)
    return out.astype(np.float32)


if __name__ == "__main__":
    rng = np.random.default_rng(0)
    x = rng.standard_normal((B, N, DIM), dtype=np.float32)
    adj = (rng.random((N, N), dtype=np.float32) < 0.05).astype(np.float32) * 0.5
    w_qkv = rng.standard_normal((DIM, 3 * F), dtype=np.float32) * DIM ** -0.5
    w_out = rng.standard_normal((F, DIM), dtype=np.float32) * F ** -0.5
    b_out = np.zeros(DIM, dtype=np.float32)
    out = kernel(x=x, adj=adj, w_qkv=w_qkv, w_out=w_out, b_out=b_out)
    print("out", out.shape, out.dtype, np.abs(out).max())
